# revision 1
# baseline (speedup 1.0000x reference)
"""DTW (dynamic time warping) distance kernel for Trainium2, 8-core SPMD.

Problem: B=32 independent (x[b] in R^{1024x64}, y[b] in R^{1024x64}) pairs.
For each pair: dist = cdist(x, y) (euclidean, [1024, 1024]); DTW dynamic
program over dist; output D[N, M] scalar per pair.

Sharding: embarrassingly parallel over batch. 8 cores x 4 batches each.

Per-core algorithm:
  Phase 1 (cdist): dist^2 = xsq_i + ysq_j - 2 x.y^T via one augmented
  matmul per [128, 512] tile (K=65: 64 feature rows of -2*x^T plus a ones
  row pairing with a ysq row); xsq added as the ACT bias of the Relu pass;
  then Sqrt. Tiles are DMAed to an internal DRAM buffer in the "DP layout"
  the DTW scan wants: [b, q, i, r] with j = 32q + r.

  Phase 2 (DTW): row recurrence cur[j] = cost[j] + min(prev[j], prev[j-1],
  cur[j-1]) vectorized via tensor_tensor_scan. Layout: partition p = 32b+q
  (batch b, column-chunk q of 32), free r = within-chunk column. Per row:
  chunk-local scans (A with init inf, B = local cumsum), chunk-end values
  hoisted to "row space" with one 32x32 stream-transpose per tile, a single
  strided-partition scan combines the 32 chunks per batch, and one
  scalar_tensor_tensor applies x = min(B + u_in, A). Cross-chunk shift for
  the diagonal term uses stream_shuffle.
"""

import numpy as np

import concourse.bass as bass
import concourse.bacc as bacc
import concourse.mybir as mybir
from concourse.tile import TileContext
from concourse.masks import make_identity
from concourse import bass_utils

f32 = mybir.dt.float32
ADD = mybir.AluOpType.add
MIN = mybir.AluOpType.min
MAX = mybir.AluOpType.max
MULT = mybir.AluOpType.mult
ACT = mybir.ActivationFunctionType

N_CORES = 8
NB = 4          # batches per core
N = 1024        # rows (x length)
M = 1024        # cols (y length)
F = 64          # features
NQ = 32         # column chunks
CL = 32         # chunk length (NQ*CL == M)
BIG = 3.0e38    # finite stand-in for +inf
SHIFT1 = [0] + list(range(31))  # stream_shuffle: out[m] = in[m-1] per 32-block


def _emit_cdist(nc, sb, ps, psmm, x_d, y_d, dist_tiles, n_rows):
    """Emit phase 1. dist_tiles: list of DRAM tiles [NB, NQ, 128, CL], one
    per 128-row block of the DP."""
    n_ti = n_rows // 128

    ident = sb.tile([128, 128], f32)
    make_identity(nc, ident[:])
    ones64 = sb.tile([64, 1], f32)
    nc.vector.memset(ones64[:], 1.0)

    XTA, YTA, XSQ = [], [], []
    for b in range(NB):
        XN = sb.tile([128, 8 * F], f32, tag="XN", bufs=2)
        YN = sb.tile([128, 8 * F], f32, tag="YN", bufs=2)
        xta = sb.tile([65, N], f32, tag=f"XTA{b}")
        yta = sb.tile([65, M], f32, tag=f"YTA{b}")
        xsq = sb.tile([128, 8], f32, tag=f"XSQ{b}")
        ysqel = sb.tile([64, M], f32, tag="YSQel", bufs=2)
        sqs = sb.tile([128, F], f32, tag="sqs", bufs=2)

        # natural-layout loads: partition = i%128, free = (i//128, f).
        # SWDGE (gpsimd) keeps these on one queue -> one wait at consumers.
        nc.gpsimd.dma_start(
            XN[:], bass.AP(x_d, b * N * F, [[F, 128], [128 * F, 8], [1, F]])
        )
        nc.gpsimd.dma_start(
            YN[:], bass.AP(y_d, b * M * F, [[F, 128], [128 * F, 8], [1, F]])
        )

        # PE transposes -> feature-major; x scaled by -2 on the PSUM copy-out.
        # 4 transposes share one [64, 512] PSUM tile so each 512-col stripe of
        # XTA/YTA has a single ACT producer (keeps matmul wait counts low).
        for g in range(2):
            pt = ps.tile([64, 512], f32, tag="pt")
            for tt in range(4):
                t = 4 * g + tt
                nc.tensor.transpose(
                    pt[:, tt * 128 : (tt + 1) * 128],
                    YN[:, t * F : (t + 1) * F], ident[:],
                )
            nc.scalar.activation(yta[0:64, g * 512 : (g + 1) * 512], pt[:], ACT.Copy)
        for g in range(max(1, n_ti // 4)):
            pt = ps.tile([64, 512], f32, tag="pt")
            nt = min(4, n_ti - 4 * g)
            for tt in range(nt):
                t = 4 * g + tt
                nc.tensor.transpose(
                    pt[:, tt * 128 : (tt + 1) * 128],
                    XN[:, t * F : (t + 1) * F], ident[:],
                )
            nc.scalar.activation(
                xta[0:64, g * 512 : g * 512 + nt * 128],
                pt[:, 0 : nt * 128], ACT.Copy, scale=-2.0,
            )
        # xsq[i] per i-tile column (ACT Square with accumulate)
        for t in range(n_ti):
            nc.scalar.activation(
                sqs[:], XN[:, t * F : (t + 1) * F], ACT.Square,
                accum_out=xsq[:, t : t + 1],
            )
        # augmented rows: xta row 64 = ones; yta row 64 = ysq
        nc.vector.memset(xta[64:65, :], 1.0)
        nc.gpsimd.tensor_tensor(ysqel[:], yta[0:64, :], yta[0:64, :], MULT)
        for nj in range(2):
            py = ps.tile([1, 512], f32, tag="py")
            nc.tensor.matmul(
                py[:], ones64[:], ysqel[:, nj * 512 : (nj + 1) * 512],
                start=True, stop=True,
            )
            nc.scalar.activation(
                yta[64:65, nj * 512 : (nj + 1) * 512], py[:], ACT.Copy
            )
        XTA.append(xta)
        YTA.append(yta)
        XSQ.append(xsq)

    # dist tiles: matmul + relu(+xsq bias) + sqrt + DMA out in DP layout.
    # One DMA-out per (ti, b) from a consolidated [128, 1024] tile keeps the
    # producer count of each dist_tiles[ti] low (4 DMAs).
    for ti in range(n_ti):
        for b in range(NB):
            ds2 = sb.tile([128, 1024], f32, tag="DS2", bufs=2)
            for nj in range(2):
                pq = psmm.tile([128, 512], f32, tag="pq")
                nc.tensor.matmul(
                    pq[:],
                    XTA[b][:, ti * 128 : (ti + 1) * 128],
                    YTA[b][:, nj * 512 : (nj + 1) * 512],
                    start=True, stop=True,
                )
                ds = sb.tile([128, 512], f32, tag="DS", bufs=3)
                nc.scalar.activation(
                    ds[:], pq[:], ACT.Relu, bias=XSQ[b][:, ti : ti + 1]
                )
                nc.scalar.activation(
                    ds2[:, nj * 512 : (nj + 1) * 512], ds[:], ACT.Sqrt
                )
            # -> dist_tiles[ti][b, q, p, r],  j = 32q + r
            dst = bass.AP(
                dist_tiles[ti].tensor,
                b * NQ * 128 * CL,
                [[CL, 128], [128 * CL, NQ], [1, CL]],
            )
            src = bass.AP(ds2.tensor, 0, [[1024, 128], [CL, NQ], [1, CL]])
            nc.sync.dma_start(dst, src)


def _emit_dtw(nc, sb, dist_tiles, xout_d, n_rows):
    """Emit phase 2: the sequential DP over n_rows rows."""
    n_ti = n_rows // 128
    RW = 128  # rows per ring half

    ring = sb.tile([128, 2 * RW * CL], f32)
    X = sb.tile([128, CL], f32)
    Mn = sb.tile([128, CL], f32)
    AB = sb.tile([128, 96], f32)
    TPA = sb.tile([128, 32], f32)
    TPB = sb.tile([128, 32], f32)
    RBu = sb.tile([128, 33], f32)
    UC = sb.tile([128, 32], f32)
    ECOL = sb.tile([128, 1], f32)
    INJ = sb.tile([128, 1], f32)
    INFT = sb.tile([128, CL], f32)

    nc.vector.memset(INFT[:], BIG)
    nc.vector.memset(INJ[:], -BIG)
    for b in range(NB):
        nc.vector.memset(INJ[32 * b : 32 * b + 1, :], BIG)
    nc.vector.memset(RBu[:], BIG)       # col 0 must stay BIG (u-scan shift-in)
    nc.vector.memset(AB[:, 64:96], 0.0)  # pad read by end-col transpose views

    ring_pitch = 2 * RW * CL

    for blk in range(n_ti):
        # DMA 128 rows (all batches/chunks) into ring half blk%2
        half = (blk % 2) * RW * CL
        dst = bass.AP(
            ring.tensor, half, [[ring_pitch, 128], [CL, RW], [1, CL]]
        )
        src = bass.AP(
            dist_tiles[blk].tensor, 0,
            [[NQ * 128 * CL, NB], [128 * CL, NQ], [CL, RW], [1, CL]],
        )
        nc.sync.dma_start(dst, src)

        for ii in range(RW):
            i = blk * RW + ii
            CR = ring[:, half + ii * CL : half + (ii + 1) * CL]
            if i == 0:
                # first DP row: mins = [0, BIG, ...] (diag D[0,0] = 0)
                nc.vector.memset(Mn[:], BIG)
                for b in range(NB):
                    nc.vector.memset(Mn[32 * b : 32 * b + 1, 0:1], 0.0)
            else:
                nc.vector.stream_shuffle(ECOL[:], X[:, CL - 1 : CL], SHIFT1)
                nc.vector.scalar_tensor_tensor(
                    Mn[:, 0:1], ECOL[:], INJ[:, 0:1], X[:, 0:1], MAX, MIN
                )
                nc.vector.tensor_tensor(
                    Mn[:, 1:CL], X[:, 1:CL], X[:, 0 : CL - 1], MIN
                )
            # chunk-local scans: A (DP with init inf), B (local cumsum)
            nc.vector.tensor_tensor_scan(AB[:, 0:CL], Mn[:], CR, BIG, MIN, ADD)
            nc.vector.tensor_tensor_scan(
                AB[:, CL : 2 * CL], CR, INFT[:], 0.0, ADD, MIN
            )
            # end columns -> row space (one stream-transpose each)
            nc.vector.transpose(
                TPA[:], bass.AP(AB.tensor, CL - 1, [[96, 128], [0, 32]])
            )
            nc.vector.transpose(
                TPB[:], bass.AP(AB.tensor, 2 * CL - 1, [[96, 128], [0, 32]])
            )
            # cross-chunk combine: u = min(u + B_e, A_e), one scan per batch
            # (walrus rejects partition-strided APs; starts {0,32,64,96} ok)
            for b in range(NB):
                nc.vector.tensor_tensor_scan(
                    RBu[32 * b : 32 * b + 1, 1:33],
                    TPB[32 * b : 32 * b + 1, 0:32],
                    TPA[32 * b : 32 * b + 1, 0:32],
                    BIG, ADD, MIN,
                )
            # back to column space: UC[32b+q, 0] = u_shift
            nc.vector.transpose(UC[:], RBu[:, 0:32])
            # apply: X = min(B + u_in, A)
            nc.vector.scalar_tensor_tensor(
                X[:], AB[:, CL : 2 * CL], UC[:, 0:1], AB[:, 0:CL], ADD, MIN
            )

    nc.sync.dma_start(xout_d[:], X[:])


def build_nc(n_rows=N):
    nc = bacc.Bacc()
    x_d = nc.dram_tensor("x", [NB, N, F], f32, kind="ExternalInput")
    y_d = nc.dram_tensor("y", [NB, M, F], f32, kind="ExternalInput")
    xout_d = nc.dram_tensor("xout", [128, CL], f32, kind="ExternalOutput")

    n_ti = n_rows // 128
    with TileContext(nc) as tc:
        with (
            tc.tile_pool(name="sb", bufs=1) as sb,
            tc.tile_pool(name="ps", bufs=2, space="PSUM") as ps,
            tc.tile_pool(name="psmm", bufs=4, space="PSUM") as psmm,
            tc.tile_pool(name="dr", bufs=1, space="DRAM") as dr,
        ):
            dist_tiles = [
                dr.tile([NB, NQ, 128, CL], f32, name=f"distbuf{t}")
                for t in range(n_ti)
            ]
            _emit_cdist(nc, sb, ps, psmm, x_d, y_d, dist_tiles, n_rows)
            _emit_dtw(nc, sb, dist_tiles, xout_d, n_rows)
    nc.compile()
    return nc


_NC_CACHE = {}


def _get_nc(n_rows=N):
    if n_rows not in _NC_CACHE:
        _NC_CACHE[n_rows] = build_nc(n_rows)
    return _NC_CACHE[n_rows]


def kernel(x: np.ndarray, y: np.ndarray) -> np.ndarray:
    """x, y: [32, 1024, 64] float32 -> [32] float32 of DTW distances."""
    x = np.ascontiguousarray(x, dtype=np.float32)
    y = np.ascontiguousarray(y, dtype=np.float32)
    nc = _get_nc()
    in_maps = [
        {"x": x[NB * c : NB * (c + 1)], "y": y[NB * c : NB * (c + 1)]}
        for c in range(N_CORES)
    ]
    res = bass_utils.run_bass_kernel_spmd(nc, in_maps, core_ids=list(range(N_CORES)))
    out = np.empty((N_CORES * NB,), np.float32)
    for c in range(N_CORES):
        xo = res.results[c]["xout"]
        for b in range(NB):
            out[NB * c + b] = xo[32 * b + 31, CL - 1]
    return out



# revision 4
# speedup vs baseline: 1.5582x; 1.5582x over previous
"""DTW (dynamic time warping) distance kernel for Trainium2, 8-core SPMD.

Problem: B=32 independent (x[b] in R^{1024x64}, y[b] in R^{1024x64}) pairs.
For each pair: dist = cdist(x, y) (euclidean, [1024, 1024]); DTW dynamic
program over dist; output D[N, M] scalar per pair.

Sharding: embarrassingly parallel over batch. 8 cores x 4 batches each.

Per-core algorithm:
  Phase 1 (cdist): dist^2 = xsq_i + ysq_j - 2 x.y^T via one augmented
  matmul per [128, 512] tile (K=65: 64 feature rows of -2*x^T plus a ones
  row pairing with a ysq row); xsq added as the ACT bias of the Relu pass;
  then Sqrt. Tiles are DMAed to an internal DRAM buffer in the "DP layout"
  the DTW scan wants: [b, q, i, r] with j = 32q + r. Additionally the
  per-chunk cost sums S[i, q] = sum_r dist[i, 32q+r] are computed with one
  vector reduce per [128, 1024] tile and stored to DRAM.

  Phase 2 (DTW): 6 DVE ops per DP row. Layout: partition p = 32b+q (batch
  b, column-chunk q of 32), free r = within-chunk column. Identities used:
    A[t]  = min(Mn[t], A[t-1]) + c[t]          (chunk-local DP scan)
    u_{q+1} = min(A_e[q], u_q + S_q)           (cross-chunk combine scan,
                                                computed in "row space"
                                                on every partition after a
                                                32x32 stream-transpose of
                                                the A end column)
    X[t]  = min(A[t], X[t-1] + c[t]), X[-1]=u  (row values, scan with
                                                per-partition initial)
    Mn'[t] = min(X[t-1], X[t]), X[-1] = u      (next row's entry mins; the
                                                cross-chunk neighbor IS u
                                                since costs >= 0)
  The u column (from a stream-transpose of the combine output) and X share
  one [128, 33] tile W, so Mn' is a single tensor_tensor over shifted
  slices of W.
"""

import numpy as np

import concourse.bass as bass
import concourse.bacc as bacc
import concourse.mybir as mybir
from concourse.tile import TileContext
from concourse.masks import make_identity
from concourse import bass_utils

f32 = mybir.dt.float32
ADD = mybir.AluOpType.add
MIN = mybir.AluOpType.min
MAX = mybir.AluOpType.max
MULT = mybir.AluOpType.mult
ACT = mybir.ActivationFunctionType
AXV = mybir.AxisListType

N_CORES = 8
NB = 4          # batches per core
N = 1024        # rows (x length)
M = 1024        # cols (y length)
F = 64          # features
NQ = 32         # column chunks
CL = 32         # chunk length (NQ*CL == M)
BIG = 3.0e38    # finite stand-in for +inf


def _emit_cdist(nc, sb, ps, psmm, x_d, y_d, dist_tiles, s_d, n_rows):
    """Emit phase 1. dist_tiles: list of DRAM tiles [NB, NQ, 128, CL], one
    per 128-row block of the DP. s_d: DRAM [NB, N, NQ] chunk sums."""
    n_ti = n_rows // 128

    ident = sb.tile([128, 128], f32)
    make_identity(nc, ident[:])
    ones64 = sb.tile([64, 1], f32)
    nc.vector.memset(ones64[:], 1.0)

    XTA, YTA, XSQ = [], [], []
    for b in range(NB):
        XN = sb.tile([128, 8 * F], f32, tag="XN", bufs=2)
        YN = sb.tile([128, 8 * F], f32, tag="YN", bufs=2)
        xta = sb.tile([65, N], f32, tag=f"XTA{b}")
        yta = sb.tile([65, M], f32, tag=f"YTA{b}")
        xsq = sb.tile([128, 8], f32, tag=f"XSQ{b}")
        ysqel = sb.tile([64, M], f32, tag="YSQel", bufs=2)
        sqs = sb.tile([128, F], f32, tag="sqs", bufs=2)

        # natural-layout loads: partition = i%128, free = (i//128, f).
        # SWDGE (gpsimd) keeps these on one queue -> one wait at consumers.
        nc.gpsimd.dma_start(
            XN[:], bass.AP(x_d, b * N * F, [[F, 128], [128 * F, 8], [1, F]])
        )
        nc.gpsimd.dma_start(
            YN[:], bass.AP(y_d, b * M * F, [[F, 128], [128 * F, 8], [1, F]])
        )

        # PE transposes -> feature-major; x scaled by -2 on the PSUM copy-out.
        # 4 transposes share one [64, 512] PSUM tile so each 512-col stripe of
        # XTA/YTA has a single ACT producer (keeps matmul wait counts low).
        for g in range(2):
            pt = ps.tile([64, 512], f32, tag="pt")
            for tt in range(4):
                t = 4 * g + tt
                nc.tensor.transpose(
                    pt[:, tt * 128 : (tt + 1) * 128],
                    YN[:, t * F : (t + 1) * F], ident[:],
                )
            nc.scalar.activation(yta[0:64, g * 512 : (g + 1) * 512], pt[:], ACT.Copy)
        for g in range(max(1, n_ti // 4)):
            pt = ps.tile([64, 512], f32, tag="pt")
            nt = min(4, n_ti - 4 * g)
            for tt in range(nt):
                t = 4 * g + tt
                nc.tensor.transpose(
                    pt[:, tt * 128 : (tt + 1) * 128],
                    XN[:, t * F : (t + 1) * F], ident[:],
                )
            nc.scalar.activation(
                xta[0:64, g * 512 : g * 512 + nt * 128],
                pt[:, 0 : nt * 128], ACT.Copy, scale=-2.0,
            )
        # xsq[i] per i-tile column (ACT Square with accumulate)
        for t in range(n_ti):
            nc.scalar.activation(
                sqs[:], XN[:, t * F : (t + 1) * F], ACT.Square,
                accum_out=xsq[:, t : t + 1],
            )
        # augmented rows: xta row 64 = ones; yta row 64 = ysq
        nc.vector.memset(xta[64:65, :], 1.0)
        nc.gpsimd.tensor_tensor(ysqel[:], yta[0:64, :], yta[0:64, :], MULT)
        for nj in range(2):
            py = ps.tile([1, 512], f32, tag="py")
            nc.tensor.matmul(
                py[:], ones64[:], ysqel[:, nj * 512 : (nj + 1) * 512],
                start=True, stop=True,
            )
            nc.scalar.activation(
                yta[64:65, nj * 512 : (nj + 1) * 512], py[:], ACT.Copy
            )
        XTA.append(xta)
        YTA.append(yta)
        XSQ.append(xsq)

    # dist tiles: matmul + relu(+xsq bias) + sqrt + DMA out in DP layout.
    # One DMA-out per (ti, b) from a consolidated [128, 1024] tile keeps the
    # producer count of each dist_tiles[ti] low (4 DMAs). The chunk sums
    # S[i, q] come from one 3D-AP vector reduce over the same tile.
    for ti in range(n_ti):
        for b in range(NB):
            ds2 = sb.tile([128, 1024], f32, tag="DS2", bufs=2)
            for nj in range(2):
                pq = psmm.tile([128, 512], f32, tag="pq")
                nc.tensor.matmul(
                    pq[:],
                    XTA[b][:, ti * 128 : (ti + 1) * 128],
                    YTA[b][:, nj * 512 : (nj + 1) * 512],
                    start=True, stop=True,
                )
                ds = sb.tile([128, 512], f32, tag="DS", bufs=3)
                nc.scalar.activation(
                    ds[:], pq[:], ACT.Relu, bias=XSQ[b][:, ti : ti + 1]
                )
                nc.scalar.activation(
                    ds2[:, nj * 512 : (nj + 1) * 512], ds[:], ACT.Sqrt
                )
            # chunk sums: S[i, q] = sum_r ds2[i, 32q + r]
            ssb = sb.tile([128, NQ], f32, tag="SSB", bufs=2)
            nc.vector.tensor_reduce(
                ssb[:],
                bass.AP(ds2.tensor, 0, [[1024, 128], [CL, NQ], [1, CL]]),
                axis=AXV.X, op=ADD,
            )
            nc.sync.dma_start(
                bass.AP(s_d, (b * N + ti * 128) * NQ, [[NQ, 128], [1, NQ]]),
                ssb[:],
            )
            # -> dist_tiles[ti][b, q, p, r],  j = 32q + r
            dst = bass.AP(
                dist_tiles[ti].tensor,
                b * NQ * 128 * CL,
                [[CL, 128], [128 * CL, NQ], [1, CL]],
            )
            src = bass.AP(ds2.tensor, 0, [[1024, 128], [CL, NQ], [1, CL]])
            nc.sync.dma_start(dst, src)


def _emit_dtw(nc, sb, dist_tiles, s_d, xout_d, n_rows):
    """Emit phase 2: the sequential DP over n_rows rows, 6 DVE ops each."""
    n_ti = n_rows // 128
    RW = 128  # rows per ring half

    ring = sb.tile([128, 2 * RW * CL], f32)
    rings = sb.tile([128, 2 * RW * NQ], f32)
    A = sb.tile([128, CL], f32)
    Mn = sb.tile([128, CL], f32)
    TPA = sb.tile([128, 32], f32)
    RBu = sb.tile([128, 33], f32)
    W = sb.tile([128, 33], f32)

    nc.vector.memset(RBu[:], BIG)       # col 0 must stay BIG (u-scan shift-in)
    nc.vector.memset(rings[:], 0.0)     # only partitions {32b} get real data
    # first DP row: entry mins = [0, BIG, ...] (diag D[0,0] = 0)
    nc.vector.memset(Mn[:], BIG)
    for b in range(NB):
        nc.vector.memset(Mn[32 * b : 32 * b + 1, 0:1], 0.0)

    ring_pitch = 2 * RW * CL
    rings_pitch = 2 * RW * NQ

    for blk in range(n_ti):
        # DMA 128 rows (all batches/chunks) into ring half blk%2
        half = (blk % 2) * RW * CL
        dst = bass.AP(
            ring.tensor, half, [[ring_pitch, 128], [CL, RW], [1, CL]]
        )
        src = bass.AP(
            dist_tiles[blk].tensor, 0,
            [[NQ * 128 * CL, NB], [128 * CL, NQ], [CL, RW], [1, CL]],
        )
        nc.sync.dma_start(dst, src)
        # chunk sums on partition 32b only (the only RBu rows ever read):
        # rings[32b, r*NQ + q] = S[b, blk*128 + r, q]
        half_s = (blk % 2) * RW * NQ
        dsts = bass.AP(
            rings.tensor, half_s,
            [[32 * rings_pitch, NB], [NQ, RW], [1, NQ]],
        )
        srcs = bass.AP(
            s_d, blk * 128 * NQ,
            [[N * NQ, NB], [NQ, RW], [1, NQ]],
        )
        nc.sync.dma_start(dsts, srcs)

        for ii in range(RW):
            i = blk * RW + ii
            CR = ring[:, half + ii * CL : half + (ii + 1) * CL]
            SR = rings[:, half_s + ii * NQ : half_s + (ii + 1) * NQ]
            # 1. chunk-local DP scan: A[t] = min(Mn[t], A[t-1]) + c[t]
            nc.vector.tensor_tensor_scan(A[:], Mn[:], CR, BIG, MIN, ADD)
            # 2. A end column -> row space (32x32 stream-transpose of the
            #    broadcast end column): TPA[32b+a, q] = A_e[32b+q]
            nc.vector.transpose(
                TPA[:], bass.AP(A.tensor, CL - 1, [[CL, 128], [0, 32]])
            )
            # 3. cross-chunk combine on all partitions:
            #    u_{q+1} = min(u_q + S_q, A_e[q]) -> RBu[:, 1+q]
            nc.vector.tensor_tensor_scan(
                RBu[:, 1:33], SR, TPA[:], BIG, ADD, MIN
            )
            # 4. back to column space: W[32b+q, 0] = u_q (only col 0 used)
            nc.vector.transpose(W[:, 0:32], RBu[:, 0:32])
            # 5. row values: X[t] = min(X[t-1] + c[t], A[t]), X[-1] = u
            nc.vector.tensor_tensor_scan(
                W[:, 1:33], CR, A[:], W[:, 0:1], ADD, MIN
            )
            # 6. next row's entry mins: Mn'[t] = min(X[t-1], X[t])
            if i < n_rows - 1:
                nc.vector.tensor_tensor(Mn[:], W[:, 0:CL], W[:, 1 : CL + 1], MIN)

    nc.sync.dma_start(xout_d[:], W[:])


def build_nc(n_rows=N):
    nc = bacc.Bacc()
    x_d = nc.dram_tensor("x", [NB, N, F], f32, kind="ExternalInput")
    y_d = nc.dram_tensor("y", [NB, M, F], f32, kind="ExternalInput")
    xout_d = nc.dram_tensor("xout", [128, 33], f32, kind="ExternalOutput")

    n_ti = n_rows // 128
    with TileContext(nc) as tc:
        with (
            tc.tile_pool(name="sb", bufs=1) as sb,
            tc.tile_pool(name="ps", bufs=2, space="PSUM") as ps,
            tc.tile_pool(name="psmm", bufs=4, space="PSUM") as psmm,
            tc.tile_pool(name="dr", bufs=1, space="DRAM") as dr,
        ):
            dist_tiles = [
                dr.tile([NB, NQ, 128, CL], f32, name=f"distbuf{t}")
                for t in range(n_ti)
            ]
            s_dt = dr.tile([NB, N, NQ], f32, name="sbuf_s")
            _emit_cdist(nc, sb, ps, psmm, x_d, y_d, dist_tiles, s_dt.tensor, n_rows)
            _emit_dtw(nc, sb, dist_tiles, s_dt.tensor, xout_d, n_rows)
    nc.compile()
    return nc


_NC_CACHE = {}


def _get_nc(n_rows=N):
    if n_rows not in _NC_CACHE:
        _NC_CACHE[n_rows] = build_nc(n_rows)
    return _NC_CACHE[n_rows]


def _make_in_maps(x, y):
    return [
        {"x": np.ascontiguousarray(x[NB * c : NB * (c + 1)]),
         "y": np.ascontiguousarray(y[NB * c : NB * (c + 1)])}
        for c in range(N_CORES)
    ]


def _extract_out(results):
    out = np.empty((N_CORES * NB,), np.float32)
    for c in range(N_CORES):
        xo = results[c]["xout"]
        for b in range(NB):
            # X[r] = W[:, 1 + r]; answer = X[chunk 31, r = 31] = W[32b+31, 32]
            out[NB * c + b] = xo[32 * b + 31, 32]
    return out


def kernel(x: np.ndarray, y: np.ndarray) -> np.ndarray:
    """x, y: [32, 1024, 64] float32 -> [32] float32 of DTW distances."""
    x = np.ascontiguousarray(x, dtype=np.float32)
    y = np.ascontiguousarray(y, dtype=np.float32)
    nc = _get_nc()
    res = bass_utils.run_bass_kernel_spmd(
        nc, _make_in_maps(x, y), core_ids=list(range(N_CORES))
    )
    return _extract_out(res.results)


# revision 9
# speedup vs baseline: 2.0988x; 1.3470x over previous
"""DTW (dynamic time warping) distance kernel for Trainium2, 8-core SPMD.

Problem: B=32 independent (x[b] in R^{1024x64}, y[b] in R^{1024x64}) pairs.
For each pair: dist = cdist(x, y) (euclidean, [1024, 1024]); DTW dynamic
program over dist; output D[N, M] scalar per pair.

Sharding: embarrassingly parallel over batch. 8 cores x 4 batches each.

Per-core algorithm:
  Phase 1 (cdist): dist^2 = xsq_i + ysq_j - 2 x.y^T via one augmented
  matmul per [128, 512] tile (K=65: 64 feature rows of -2*x^T plus a ones
  row pairing with a ysq row); xsq added as the ACT bias of the Relu pass;
  then Sqrt. Tiles are DMAed to one DRAM buffer in 32x32-tile-blocked
  layout dist[b][I][J][r][t] (i = 32I + r, j = 32J + t).

  Phase 2 (DTW): tile-wavefront DP, 2 DVE ops per row-step. The [32, 32]
  tile grid is swept along anti-diagonals w = I + J (63 waves); partition
  p = 32b + I owns tile row I and processes tile (I, w - I) during wave w.
  Within a tile, each of the 32 rows is ONE chunk-free scan:
      X_r[t] = min(Mn_r[t], X_r[t-1]) + c[t],   X_r[-1] = L_r
      Mn_r[t] = min(W_{r-1}[t], W_{r-1}[t+1])   (one tensor_tensor)
  where W packs [L_r, X_r[0..31]] per row at pitch 33, so the left
  boundary L_r (right column of the west tile, same partition, previous
  wave) enters as the scan's per-partition initial, and the top boundary
  (bottom row + corner of the north tile) moves down one partition via a
  single stream_shuffle of the previous wave's last W row slot. Inactive
  partitions stay at BIG automatically: their W starts BIG and
  min(BIG, BIG + c) = BIG for any cost c >= 0 (ring memset to 0).
"""

import numpy as np

import concourse.bass as bass
import concourse.bacc as bacc
import concourse.mybir as mybir
from concourse.tile import TileContext
from concourse.masks import make_identity
from concourse import bass_utils

f32 = mybir.dt.float32
ADD = mybir.AluOpType.add
MIN = mybir.AluOpType.min
MAX = mybir.AluOpType.max
MULT = mybir.AluOpType.mult
ACT = mybir.ActivationFunctionType

N_CORES = 8
NB = 4          # batches per core
N = 1024        # rows (x length)
M = 1024        # cols (y length)
F = 64          # features
T = 32          # DP tile edge
G = 32          # tile grid edge (G*T == N == M)
NW = 2 * G - 1  # waves
BIG = 3.0e38    # finite stand-in for +inf
SHIFT1 = [0] + list(range(31))  # stream_shuffle: out[m] = in[m-1] per 32-block

BSZ = G * G * T * T       # dist elements per batch (1 Mi)
ISZ = G * T * T           # dist elements per tile row I (32 Ki)


def _emit_cdist(nc, sb, ps, psmm, x_d, y_d, dist_d, n_rows):
    """Emit phase 1. dist_d: DRAM [NB * BSZ], tile-blocked layout."""
    n_ti = n_rows // 128

    ident = sb.tile([128, 128], f32)
    make_identity(nc, ident[:])
    ones64 = sb.tile([64, 1], f32)
    nc.vector.memset(ones64[:], 1.0)

    XTA, YTA, XSQ = [], [], []
    for b in range(NB):
        XN = sb.tile([128, 8 * F], f32, tag="XN", bufs=2)
        YN = sb.tile([128, 8 * F], f32, tag="YN", bufs=2)
        xta = sb.tile([65, N], f32, tag=f"XTA{b}")
        yta = sb.tile([65, M], f32, tag=f"YTA{b}")
        xsq = sb.tile([128, 8], f32, tag=f"XSQ{b}")
        ysqel = sb.tile([64, M], f32, tag="YSQel", bufs=2)
        sqs = sb.tile([128, F], f32, tag="sqs", bufs=2)

        # natural-layout loads: partition = i%128, free = (i//128, f).
        nc.gpsimd.dma_start(
            XN[:], bass.AP(x_d, b * N * F, [[F, 128], [128 * F, 8], [1, F]])
        )
        nc.gpsimd.dma_start(
            YN[:], bass.AP(y_d, b * M * F, [[F, 128], [128 * F, 8], [1, F]])
        )

        # PE transposes -> feature-major; x scaled by -2 on the PSUM copy-out.
        for g in range(2):
            pt = ps.tile([64, 512], f32, tag="pt")
            for tt in range(4):
                t = 4 * g + tt
                nc.tensor.transpose(
                    pt[:, tt * 128 : (tt + 1) * 128],
                    YN[:, t * F : (t + 1) * F], ident[:],
                )
            nc.scalar.activation(yta[0:64, g * 512 : (g + 1) * 512], pt[:], ACT.Copy)
        for g in range(max(1, n_ti // 4)):
            pt = ps.tile([64, 512], f32, tag="pt")
            nt = min(4, n_ti - 4 * g)
            for tt in range(nt):
                t = 4 * g + tt
                nc.tensor.transpose(
                    pt[:, tt * 128 : (tt + 1) * 128],
                    XN[:, t * F : (t + 1) * F], ident[:],
                )
            nc.scalar.activation(
                xta[0:64, g * 512 : g * 512 + nt * 128],
                pt[:, 0 : nt * 128], ACT.Copy, scale=-2.0,
            )
        # xsq[i] per i-tile column (ACT Square with accumulate)
        for t in range(n_ti):
            nc.scalar.activation(
                sqs[:], XN[:, t * F : (t + 1) * F], ACT.Square,
                accum_out=xsq[:, t : t + 1],
            )
        # augmented rows: xta row 64 = ones; yta row 64 = ysq
        nc.vector.memset(xta[64:65, :], 1.0)
        nc.gpsimd.tensor_tensor(ysqel[:], yta[0:64, :], yta[0:64, :], MULT)
        for nj in range(2):
            py = ps.tile([1, 512], f32, tag="py")
            nc.tensor.matmul(
                py[:], ones64[:], ysqel[:, nj * 512 : (nj + 1) * 512],
                start=True, stop=True,
            )
            nc.scalar.activation(
                yta[64:65, nj * 512 : (nj + 1) * 512], py[:], ACT.Copy
            )
        XTA.append(xta)
        YTA.append(yta)
        XSQ.append(xsq)

    # dist tiles: matmul + relu(+xsq bias) + sqrt + DMA out tile-blocked.
    for ti in range(n_ti):
        for b in range(NB):
            ds2 = sb.tile([128, 1024], f32, tag="DS2", bufs=2)
            for nj in range(2):
                pq = psmm.tile([128, 512], f32, tag="pq")
                nc.tensor.matmul(
                    pq[:],
                    XTA[b][:, ti * 128 : (ti + 1) * 128],
                    YTA[b][:, nj * 512 : (nj + 1) * 512],
                    start=True, stop=True,
                )
                ds = sb.tile([128, 512], f32, tag="DS", bufs=3)
                nc.scalar.activation(
                    ds[:], pq[:], ACT.Relu, bias=XSQ[b][:, ti : ti + 1]
                )
                nc.scalar.activation(
                    ds2[:, nj * 512 : (nj + 1) * 512], ds[:], ACT.Sqrt
                )
            # -> dist_d[b][I][J][r][t] with I = 4 ti + Ii, i_local = 32 Ii + r
            for Ii in range(4):
                dst = bass.AP(
                    dist_d, b * BSZ + (ti * 4 + Ii) * ISZ,
                    [[T, T], [T * T, G], [1, T]],
                )
                src = bass.AP(
                    ds2.tensor, Ii * 32 * 1024,
                    [[1024, T], [T, G], [1, T]],
                )
                nc.sync.dma_start(dst, src)


def _emit_dtw(nc, sb, dist_d, xout_d):
    """Emit phase 2: 63-wave tile wavefront, 2 DVE ops per row-step."""
    NSLOT = 4
    ring = sb.tile([128, NSLOT * T * T], f32)
    Wab = [
        sb.tile([128, 33 * T], f32, tag=f"W{k}", name=f"Wab{k}")
        for k in range(2)
    ]
    TOPr = sb.tile([128, 33], f32)
    TOPf = sb.tile([128, 33], f32)
    Mn = sb.tile([128, T], f32)
    INJ = sb.tile([128, 1], f32)

    ring_pitch = NSLOT * T * T
    w_pitch = 33 * T

    nc.vector.memset(ring[:], 0.0)      # inactive lanes see costs >= 0
    for k in range(2):
        nc.vector.memset(Wab[k][:], BIG)
    nc.vector.memset(INJ[:], -BIG)
    for b in range(NB):                 # I = 0 lanes: top boundary is BIG
        nc.vector.memset(INJ[32 * b : 32 * b + 1, :], BIG)

    for w in range(NW):
        Wc = Wab[w % 2]
        Wp = Wab[(w + 1) % 2]
        slot = (w % NSLOT) * T * T

        # diagonal load: partition 32b + I gets tile (I, w - I), I active
        ilo = max(0, w - (G - 1))
        ihi = min(G - 1, w)
        cnt = ihi - ilo + 1
        for b in range(NB):
            dst = bass.AP(
                ring.tensor, (32 * b + ilo) * ring_pitch + slot,
                [[ring_pitch, cnt], [1, T * T]],
            )
            src = bass.AP(
                dist_d, b * BSZ + ilo * ISZ + (w - ilo) * T * T,
                [[ISZ - T * T, cnt], [1, T * T]],
            )
            nc.gpsimd.dma_start(dst, src)

        # top boundary: [corner, bottom row] of the north tile, one
        # partition down. TOPr[32b] is garbage -> forced BIG via INJ.
        nc.vector.stream_shuffle(TOPr[:], Wp[:, 33 * (T - 1) : 33 * T], SHIFT1)
        nc.vector.scalar_tensor_tensor(
            TOPf[:], TOPr[:], INJ[:, 0:1], TOPr[:], MAX, MAX
        )
        if w == 0:
            for b in range(NB):         # D[0,0] corner
                nc.vector.memset(TOPf[32 * b : 32 * b + 1, 0:1], 0.0)

        # left boundary: L_r = right column of the west tile (same
        # partition, previous wave) -> W slot column r*33
        nc.vector.tensor_copy(
            bass.AP(Wc.tensor, 0, [[w_pitch, 128], [33, T]]),
            bass.AP(Wp.tensor, T, [[w_pitch, 128], [33, T]]),
        )

        for r in range(T):
            if r == 0:
                lo, hi = TOPf[:, 0:T], TOPf[:, 1 : T + 1]
            else:
                base = (r - 1) * 33
                lo = Wc[:, base : base + T]
                hi = Wc[:, base + 1 : base + T + 1]
            nc.vector.tensor_tensor(Mn[:], lo, hi, MIN)
            nc.vector.tensor_tensor_scan(
                Wc[:, r * 33 + 1 : r * 33 + T + 1],
                Mn[:],
                ring[:, slot + r * T : slot + (r + 1) * T],
                Wc[:, r * 33 : r * 33 + 1],
                MIN, ADD,
            )

    nc.sync.dma_start(xout_d[:], Wab[(NW - 1) % 2][:, 33 * T - 1 : 33 * T])


def build_nc(n_rows=N):
    nc = bacc.Bacc()
    x_d = nc.dram_tensor("x", [NB, N, F], f32, kind="ExternalInput")
    y_d = nc.dram_tensor("y", [NB, M, F], f32, kind="ExternalInput")
    xout_d = nc.dram_tensor("xout", [128, 1], f32, kind="ExternalOutput")

    with TileContext(nc) as tc:
        with (
            tc.tile_pool(name="sb", bufs=1) as sb,
            tc.tile_pool(name="ps", bufs=2, space="PSUM") as ps,
            tc.tile_pool(name="psmm", bufs=4, space="PSUM") as psmm,
            tc.tile_pool(name="dr", bufs=1, space="DRAM") as dr,
        ):
            dist_t = dr.tile([NB * BSZ], f32, name="distbuf")
            _emit_cdist(nc, sb, ps, psmm, x_d, y_d, dist_t.tensor, n_rows)
            _emit_dtw(nc, sb, dist_t.tensor, xout_d)
    nc.compile()
    return nc


_NC_CACHE = {}


def _get_nc(n_rows=N):
    if n_rows not in _NC_CACHE:
        _NC_CACHE[n_rows] = build_nc(n_rows)
    return _NC_CACHE[n_rows]


def _make_in_maps(x, y):
    return [
        {"x": np.ascontiguousarray(x[NB * c : NB * (c + 1)]),
         "y": np.ascontiguousarray(y[NB * c : NB * (c + 1)])}
        for c in range(N_CORES)
    ]


def _extract_out(results):
    out = np.empty((N_CORES * NB,), np.float32)
    for c in range(N_CORES):
        xo = results[c]["xout"]
        for b in range(NB):
            out[NB * c + b] = xo[32 * b + 31, 0]
    return out


def kernel(x: np.ndarray, y: np.ndarray) -> np.ndarray:
    """x, y: [32, 1024, 64] float32 -> [32] float32 of DTW distances."""
    x = np.ascontiguousarray(x, dtype=np.float32)
    y = np.ascontiguousarray(y, dtype=np.float32)
    nc = _get_nc()
    res = bass_utils.run_bass_kernel_spmd(
        nc, _make_in_maps(x, y), core_ids=list(range(N_CORES))
    )
    return _extract_out(res.results)


# revision 10
# speedup vs baseline: 2.3172x; 1.1041x over previous
"""DTW (dynamic time warping) distance kernel for Trainium2, 8-core SPMD.

Problem: B=32 independent (x[b] in R^{1024x64}, y[b] in R^{1024x64}) pairs.
For each pair: dist = cdist(x, y) (euclidean, [1024, 1024]); DTW dynamic
program over dist; output D[N, M] scalar per pair.

Sharding: embarrassingly parallel over batch. 8 cores x 4 batches each.

Per-core algorithm:
  Phase 1 (cdist): dist^2 = xsq_i + ysq_j - 2 x.y^T via one augmented
  matmul per [128, 512] tile (K=65: 64 feature rows of -2*x^T plus a ones
  row pairing with a ysq row); xsq added as the ACT bias of the Relu pass;
  then Sqrt. Tiles are DMAed to one DRAM buffer in 32x32-tile-blocked
  layout dist[b][I][J][r][t] (i = 32I + r, j = 32J + t).

  Phase 2 (DTW): tile-wavefront DP, 2 DVE ops per row-step. The [32, 32]
  tile grid is swept along anti-diagonals w = I + J (63 waves); partition
  p = 32b + I owns tile row I and processes tile (I, w - I) during wave w.
  Within a tile, each of the 32 rows is ONE chunk-free scan:
      X_r[t] = min(Mn_r[t], X_r[t-1]) + c[t],   X_r[-1] = L_r
      Mn_r[t] = min(W_{r-1}[t], W_{r-1}[t+1])   (one tensor_tensor)
  where W packs [L_r, X_r[0..31]] per row at pitch 33, so the left
  boundary L_r (right column of the west tile, same partition, previous
  wave) enters as the scan's per-partition initial, and the top boundary
  (bottom row + corner of the north tile) moves down one partition via a
  single stream_shuffle of the previous wave's last W row slot. Inactive
  partitions stay at BIG automatically: their W starts BIG and
  min(BIG, BIG + c) = BIG for any cost c >= 0 (ring memset to 0).
"""

import numpy as np

import concourse.bass as bass
import concourse.bacc as bacc
import concourse.mybir as mybir
from concourse.tile import TileContext
from concourse.masks import make_identity
from concourse import bass_utils

f32 = mybir.dt.float32
ADD = mybir.AluOpType.add
MIN = mybir.AluOpType.min
MAX = mybir.AluOpType.max
MULT = mybir.AluOpType.mult
ACT = mybir.ActivationFunctionType

N_CORES = 8
NB = 4          # batches per core
N = 1024        # rows (x length)
M = 1024        # cols (y length)
F = 64          # features
T = 32          # DP tile edge
G = 32          # tile grid edge (G*T == N == M)
NW = 2 * G - 1  # waves
BIG = 3.0e38    # finite stand-in for +inf
SHIFT1 = [0] + list(range(31))  # stream_shuffle: out[m] = in[m-1] per 32-block

BSZ = G * G * T * T       # dist elements per batch (1 Mi)
ISZ = G * T * T           # dist elements per tile row I (32 Ki)


def _emit_cdist(nc, sb, ps, psmm, x_d, y_d, dist_d, n_rows):
    """Emit phase 1. dist_d: DRAM [NB * BSZ], tile-blocked layout."""
    n_ti = n_rows // 128

    ident = sb.tile([128, 128], f32)
    make_identity(nc, ident[:])
    ones64 = sb.tile([64, 1], f32)
    nc.vector.memset(ones64[:], 1.0)

    XTA, YTA, XSQ = [], [], []
    for b in range(NB):
        XN = sb.tile([128, 8 * F], f32, tag="XN", bufs=2)
        YN = sb.tile([128, 8 * F], f32, tag="YN", bufs=2)
        xta = sb.tile([65, N], f32, tag=f"XTA{b}")
        yta = sb.tile([65, M], f32, tag=f"YTA{b}")
        xsq = sb.tile([128, 8], f32, tag=f"XSQ{b}")
        ysqel = sb.tile([64, M], f32, tag="YSQel", bufs=2)
        sqs = sb.tile([128, F], f32, tag="sqs", bufs=2)

        # natural-layout loads: partition = i%128, free = (i//128, f).
        nc.gpsimd.dma_start(
            XN[:], bass.AP(x_d, b * N * F, [[F, 128], [128 * F, 8], [1, F]])
        )
        nc.gpsimd.dma_start(
            YN[:], bass.AP(y_d, b * M * F, [[F, 128], [128 * F, 8], [1, F]])
        )

        # PE transposes -> feature-major; x scaled by -2 on the PSUM copy-out.
        for g in range(2):
            pt = ps.tile([64, 512], f32, tag="pt")
            for tt in range(4):
                t = 4 * g + tt
                nc.tensor.transpose(
                    pt[:, tt * 128 : (tt + 1) * 128],
                    YN[:, t * F : (t + 1) * F], ident[:],
                )
            nc.scalar.activation(yta[0:64, g * 512 : (g + 1) * 512], pt[:], ACT.Copy)
        for g in range(max(1, n_ti // 4)):
            pt = ps.tile([64, 512], f32, tag="pt")
            nt = min(4, n_ti - 4 * g)
            for tt in range(nt):
                t = 4 * g + tt
                nc.tensor.transpose(
                    pt[:, tt * 128 : (tt + 1) * 128],
                    XN[:, t * F : (t + 1) * F], ident[:],
                )
            nc.scalar.activation(
                xta[0:64, g * 512 : g * 512 + nt * 128],
                pt[:, 0 : nt * 128], ACT.Copy, scale=-2.0,
            )
        # xsq[i] per i-tile column (ACT Square with accumulate)
        for t in range(n_ti):
            nc.scalar.activation(
                sqs[:], XN[:, t * F : (t + 1) * F], ACT.Square,
                accum_out=xsq[:, t : t + 1],
            )
        # augmented rows: xta row 64 = ones; yta row 64 = ysq
        nc.vector.memset(xta[64:65, :], 1.0)
        nc.gpsimd.tensor_tensor(ysqel[:], yta[0:64, :], yta[0:64, :], MULT)
        for nj in range(2):
            py = ps.tile([1, 512], f32, tag="py")
            nc.tensor.matmul(
                py[:], ones64[:], ysqel[:, nj * 512 : (nj + 1) * 512],
                start=True, stop=True,
            )
            nc.scalar.activation(
                yta[64:65, nj * 512 : (nj + 1) * 512], py[:], ACT.Copy
            )
        XTA.append(xta)
        YTA.append(yta)
        XSQ.append(xsq)

    # dist tiles: matmul + relu(+xsq bias) + sqrt + DMA out tile-blocked.
    for ti in range(n_ti):
        for b in range(NB):
            ds2 = sb.tile([128, 1024], f32, tag="DS2", bufs=2)
            for nj in range(2):
                pq = psmm.tile([128, 512], f32, tag="pq")
                nc.tensor.matmul(
                    pq[:],
                    XTA[b][:, ti * 128 : (ti + 1) * 128],
                    YTA[b][:, nj * 512 : (nj + 1) * 512],
                    start=True, stop=True,
                )
                ds = sb.tile([128, 512], f32, tag="DS", bufs=3)
                nc.scalar.activation(
                    ds[:], pq[:], ACT.Relu, bias=XSQ[b][:, ti : ti + 1]
                )
                nc.scalar.activation(
                    ds2[:, nj * 512 : (nj + 1) * 512], ds[:], ACT.Sqrt
                )
            # -> dist_d[b][I][J][r][t] with I = 4 ti + Ii, i_local = 32 Ii + r
            for Ii in range(4):
                dst = bass.AP(
                    dist_d, b * BSZ + (ti * 4 + Ii) * ISZ,
                    [[T, T], [T * T, G], [1, T]],
                )
                src = bass.AP(
                    ds2.tensor, Ii * 32 * 1024,
                    [[1024, T], [T, G], [1, T]],
                )
                nc.sync.dma_start(dst, src)


def _emit_dtw(nc, sb, dist_d, xout_d):
    """Emit phase 2: 63-wave tile wavefront, 2 DVE ops per row-step.

    Row slots are 33 wide everywhere: W rows pack [L_r, X_r[0..31]], the
    Mn tile packs [L_r, Mn_r[0..31]], and ring rows pack [0, c_r[0..31]].
    The X scan runs over all 33 with constant initial BIG: its first step
    computes min(L_r, BIG) + 0 = L_r, writing the L column of W back in
    place (avoids the slower per-partition-AP initial form).
    """
    NSLOT = 4
    RP = 33 * T                         # row-slot pitch (one wave of rows)
    ring = sb.tile([128, NSLOT * RP], f32)
    Wab = [
        sb.tile([128, RP], f32, tag=f"W{k}", name=f"Wab{k}")
        for k in range(2)
    ]
    MnE = sb.tile([128, RP], f32)
    TOPr = sb.tile([128, 33], f32)
    TOPf = sb.tile([128, 33], f32)
    INJ = sb.tile([128, 1], f32)

    ring_pitch = NSLOT * RP

    nc.vector.memset(ring[:], 0.0)      # inactive lanes see costs >= 0
    for k in range(2):
        nc.vector.memset(Wab[k][:], BIG)
    nc.vector.memset(INJ[:], -BIG)
    for b in range(NB):                 # I = 0 lanes: top boundary is BIG
        nc.vector.memset(INJ[32 * b : 32 * b + 1, :], BIG)

    for w in range(NW):
        Wc = Wab[w % 2]
        Wp = Wab[(w + 1) % 2]
        slot = (w % NSLOT) * RP

        # diagonal load: partition 32b + I gets tile (I, w - I), I active;
        # row r of the tile lands at ring[:, slot + r*33 + 1 :+32] (col
        # r*33 stays 0 from the initial memset).
        ilo = max(0, w - (G - 1))
        ihi = min(G - 1, w)
        cnt = ihi - ilo + 1
        for b in range(NB):
            dst = bass.AP(
                ring.tensor, (32 * b + ilo) * ring_pitch + slot + 1,
                [[ring_pitch, cnt], [33, T], [1, T]],
            )
            src = bass.AP(
                dist_d, b * BSZ + ilo * ISZ + (w - ilo) * T * T,
                [[ISZ - T * T, cnt], [T, T], [1, T]],
            )
            nc.gpsimd.dma_start(dst, src)

        # top boundary: [corner, bottom row] of the north tile, one
        # partition down. TOPr[32b] is garbage -> forced BIG via INJ.
        nc.vector.stream_shuffle(TOPr[:], Wp[:, RP - 33 : RP], SHIFT1)
        nc.vector.scalar_tensor_tensor(
            TOPf[:], TOPr[:], INJ[:, 0:1], TOPr[:], MAX, MAX
        )
        if w == 0:
            for b in range(NB):         # D[0,0] corner
                nc.vector.memset(TOPf[32 * b : 32 * b + 1, 0:1], 0.0)

        # left boundary: L_r = right column of the west tile (same
        # partition, previous wave) -> MnE column r*33
        nc.vector.tensor_copy(
            bass.AP(MnE.tensor, 0, [[RP, 128], [33, T]]),
            bass.AP(Wp.tensor, T, [[RP, 128], [33, T]]),
        )

        for r in range(T):
            if r == 0:
                lo, hi = TOPf[:, 0:T], TOPf[:, 1 : T + 1]
            else:
                base = (r - 1) * 33
                lo = Wc[:, base : base + T]
                hi = Wc[:, base + 1 : base + T + 1]
            nc.vector.tensor_tensor(MnE[:, r * 33 + 1 : r * 33 + 33], lo, hi, MIN)
            nc.vector.tensor_tensor_scan(
                Wc[:, r * 33 : r * 33 + 33],
                MnE[:, r * 33 : r * 33 + 33],
                ring[:, slot + r * 33 : slot + r * 33 + 33],
                BIG, MIN, ADD,
            )

    nc.sync.dma_start(xout_d[:], Wab[(NW - 1) % 2][:, RP - 1 : RP])


def build_nc(n_rows=N):
    nc = bacc.Bacc()
    x_d = nc.dram_tensor("x", [NB, N, F], f32, kind="ExternalInput")
    y_d = nc.dram_tensor("y", [NB, M, F], f32, kind="ExternalInput")
    xout_d = nc.dram_tensor("xout", [128, 1], f32, kind="ExternalOutput")

    with TileContext(nc) as tc:
        with (
            tc.tile_pool(name="sb", bufs=1) as sb,
            tc.tile_pool(name="ps", bufs=2, space="PSUM") as ps,
            tc.tile_pool(name="psmm", bufs=4, space="PSUM") as psmm,
            tc.tile_pool(name="dr", bufs=1, space="DRAM") as dr,
        ):
            dist_t = dr.tile([NB * BSZ], f32, name="distbuf")
            _emit_cdist(nc, sb, ps, psmm, x_d, y_d, dist_t.tensor, n_rows)
            _emit_dtw(nc, sb, dist_t.tensor, xout_d)
    nc.compile()
    return nc


_NC_CACHE = {}


def _get_nc(n_rows=N):
    if n_rows not in _NC_CACHE:
        _NC_CACHE[n_rows] = build_nc(n_rows)
    return _NC_CACHE[n_rows]


def _make_in_maps(x, y):
    return [
        {"x": np.ascontiguousarray(x[NB * c : NB * (c + 1)]),
         "y": np.ascontiguousarray(y[NB * c : NB * (c + 1)])}
        for c in range(N_CORES)
    ]


def _extract_out(results):
    out = np.empty((N_CORES * NB,), np.float32)
    for c in range(N_CORES):
        xo = results[c]["xout"]
        for b in range(NB):
            out[NB * c + b] = xo[32 * b + 31, 0]
    return out


def kernel(x: np.ndarray, y: np.ndarray) -> np.ndarray:
    """x, y: [32, 1024, 64] float32 -> [32] float32 of DTW distances."""
    x = np.ascontiguousarray(x, dtype=np.float32)
    y = np.ascontiguousarray(y, dtype=np.float32)
    nc = _get_nc()
    res = bass_utils.run_bass_kernel_spmd(
        nc, _make_in_maps(x, y), core_ids=list(range(N_CORES))
    )
    return _extract_out(res.results)
